# revision 1
# baseline (speedup 1.0000x reference)
"""Bipartite 2-layer GraphSAGE encoder on 8 Trainium2 NeuronCores.

Strategy (no data-dependent DMA on device — all edge irregularity is
resolved on the host into dense, statically-addressed layouts):

  reference:
    xs  = x_site @ Wsi + bsi ; xv = x_vendor @ Wvi + bvi
    xv1 = relu(mean_{dst}(xs[src]) @ Wl1sv + bl1sv + xv @ Wr1sv)
    xs1 = relu(mean_{src}(xv[dst]) @ Wl1vs + bl1vs + xs @ Wr1vs)
    xv2 = relu(mean_{dst}(xs1[src]) @ Wl2sv + bl2sv + xv1 @ Wr2sv)
    xs2 = relu(mean_{src}(xv1[dst]) @ Wl2vs + bl2vs + xs1 @ Wr2vs)

  Folding: with m9 = mean_{src}(x_vendor[dst]) (per site) and
  m10 = mean_{dst}(x_site[src]) (per vendor) — both pure index/input
  preprocessing — every layer-1 activation is an affine map of the raw
  19-dim concatenated features:
    xs1[i] = relu([m9[i] | x_site[i]] @ W1s + h1s)
    xv1[j] = relu([m10[j] | x_vendor[j]] @ W1v + h1v)
  so the layer-2 messages xs1[src_e] / xv1[dst_e] are computed on-device
  from host-pre-gathered per-edge 19-dim features (the "gathered edge
  features" of the sharding hint), then segment-summed per destination
  with identity-matmul PSUM accumulation. Edges are sharded by owner
  (vendor-range for the site->vendor direction, site-range for
  vendor->site), so no collectives are needed.

  Per-edge slot layout ("ELL"): owners are globally degree-sorted and
  dealt round-robin to the 8 cores; within a core, tiles of 128 owners
  padded to the tile-max degree. Slot-block (tile, j) = one [20, 128]
  bf16 matrix (19 features + validity row; pad slots get -1e6 validity
  so relu(z) == 0). Four slot-blocks are stacked vertically at
  partition offsets 0/32/64/96 and dispatched with tile_position
  row-tiling.
"""

import numpy as np
import ml_dtypes

bf16 = ml_dtypes.bfloat16

M = 8
NS, NV, E = 100000, 20000, 3200000
SITE_IN, VENDOR_IN, HID, OUT = 10, 9, 64, 32
NS_LOC, NV_LOC = NS // M, NV // M          # 12500 / 2500
NT_B = (NS_LOC + 127) // 128               # 98 site tiles per core
NT_A = (NV_LOC + 127) // 128               # 20 vendor tiles per core
NS_PAD, NV_PAD = NT_B * 128, NT_A * 128    # 12544 / 2560
KF = 20                                    # 19 features + validity row
PAD_MULT = 4
VALID_NEG = -60000.0                       # bf16-representable big negative


def _owner_maps(deg, n, m):
    order = np.argsort(-deg, kind="stable")
    owner = np.empty(n, np.int32)
    local = np.empty(n, np.int32)
    k = np.arange(n)
    owner[order] = k % m
    local[order] = (k // m).astype(np.int32)
    return owner, local


def _ell_uniform(owner, local, n_loc, n_tiles, edge_feat, m):
    """Build per-core uniform-pad ELL U arrays.

    owner/local: per-edge owner core + local owner index.
    edge_feat: [E, 19] float32 per-edge features (already gathered).
    Returns U [m, 128, ncols] bf16 (stack-4 layout), pads [n_tiles].
    """
    # per (core, local) counts
    flat = owner.astype(np.int64) * n_loc + local
    counts = np.bincount(flat, minlength=m * n_loc).reshape(m, n_loc)
    tmax = np.zeros(n_tiles, np.int64)
    for t in range(n_tiles):
        hi = min(128 * (t + 1), n_loc)
        tmax[t] = counts[:, 128 * t:hi].max()
    pads = ((np.maximum(tmax, 1) + PAD_MULT - 1) // PAD_MULT) * PAD_MULT
    nblk = int(pads.sum())                       # slot-blocks per core
    blk_off = np.concatenate([[0], np.cumsum(pads)])
    U = np.zeros((m, KF, nblk * 128), np.float32)
    U[:, 19, :] = VALID_NEG                      # validity row default: pad
    # vectorized slot fill
    sort_key = (owner.astype(np.int64) * n_loc + local)
    order = np.argsort(sort_key, kind="stable")
    so, sl = owner[order], local[order]
    sf = edge_feat[order]
    starts = np.concatenate([[0], np.cumsum(counts.reshape(-1))])
    pos = np.arange(len(order)) - starts[so.astype(np.int64) * n_loc + sl]
    t_idx = sl // 128
    p_idx = sl % 128
    blk = blk_off[t_idx] + pos                    # block index within core
    col = blk * 128 + p_idx
    for f in range(19):
        U[so, f, col] = sf[:, f]
    U[so, 19, col] = 0.0
    return U.astype(bf16), pads


def _prep(x_site, x_vendor, src, dst, W):
    src = np.asarray(src).astype(np.int64)
    dst = np.asarray(dst).astype(np.int64)
    x_site = np.asarray(x_site, np.float32)
    x_vendor = np.asarray(x_vendor, np.float32)

    deg_v = np.bincount(dst, minlength=NV)
    deg_s = np.bincount(src, minlength=NS)
    rv = (1.0 / np.maximum(deg_v, 1)).astype(np.float32)
    rs = (1.0 / np.maximum(deg_s, 1)).astype(np.float32)

    agg10 = np.zeros((NV, SITE_IN), np.float32)
    np.add.at(agg10, dst, x_site[src])
    mean10 = agg10 * rv[:, None]
    agg9 = np.zeros((NS, VENDOR_IN), np.float32)
    np.add.at(agg9, src, x_vendor[dst])
    mean9 = agg9 * rs[:, None]

    v_owner, v_local = _owner_maps(deg_v, NV, M)
    s_owner, s_local = _owner_maps(deg_s, NS, M)

    # ordering A: sharded by dst owner; edge features -> xs1[src] inputs
    featA = np.concatenate([mean9[src], x_site[src]], axis=1)
    U_A, padsA = _ell_uniform(v_owner[dst], v_local[dst], NV_LOC, NT_A, featA, M)
    # ordering B: sharded by src owner; features -> xv1[dst] inputs
    featB = np.concatenate([mean10[dst], x_vendor[dst]], axis=1)
    U_B, padsB = _ell_uniform(s_owner[src], s_local[src], NS_LOC, NT_B, featB, M)

    # folded layer-1 weights (KF rows; validity row weight 1 so pad slots
    # get z = h - 60000 < 0 -> relu 0)
    W1v = np.concatenate([
        W['W_site_in'] @ W['Wl1sv'], W['W_vendor_in'] @ W['Wr1sv'],
        np.ones((1, HID), np.float32)], axis=0).astype(np.float32)   # [20,64]
    h1v = (W['b_site_in'] @ W['Wl1sv'] + W['bl1sv']
           + W['b_vendor_in'] @ W['Wr1sv']).astype(np.float32)
    W1s = np.concatenate([
        W['W_vendor_in'] @ W['Wl1vs'], W['W_site_in'] @ W['Wr1vs'],
        np.ones((1, HID), np.float32)], axis=0).astype(np.float32)
    h1s = (W['b_vendor_in'] @ W['Wl1vs'] + W['bl1vs']
           + W['b_site_in'] @ W['Wr1vs']).astype(np.float32)

    # per-core dense node features V (19 rows + zero validity row),
    # feature-major [KF, n_pad]; and r per-tile [128, ntiles]
    Vv = np.zeros((M, KF, NV_PAD), np.float32)
    Vs = np.zeros((M, KF, NS_PAD), np.float32)
    rA = np.zeros((M, 128, NT_A), np.float32)
    rB = np.zeros((M, 128, NT_B), np.float32)
    for c in range(M):
        sel = np.flatnonzero(v_owner == c)
        loc = v_local[sel]
        Vv[c, :SITE_IN, loc] = mean10[sel]
        Vv[c, SITE_IN:19, loc] = x_vendor[sel]
        rA[c, loc % 128, loc // 128] = rv[sel]
        sel = np.flatnonzero(s_owner == c)
        loc = s_local[sel]
        Vs[c, :VENDOR_IN, loc] = mean9[sel]
        Vs[c, VENDOR_IN:19, loc] = x_site[sel]
        rB[c, loc % 128, loc // 128] = rs[sel]

    meta = dict(v_owner=v_owner, v_local=v_local,
                s_owner=s_owner, s_local=s_local)
    dev = []
    for c in range(M):
        dev.append(dict(U_A=np.ascontiguousarray(U_A[c]),
                        U_B=np.ascontiguousarray(U_B[c]),
                        Vv=Vv[c].astype(bf16), Vs=Vs[c].astype(bf16),
                        rA=rA[c], rB=rB[c]))
    shared = dict(W1v=W1v.astype(bf16), W1s=W1s.astype(bf16),
                  h1v=h1v.reshape(HID, 1), h1s=h1s.reshape(HID, 1),
                  Wl2sv=W['Wl2sv'], Wr2sv=W['Wr2sv'], bl2sv=W['bl2sv'].reshape(1, OUT),
                  Wl2vs=W['Wl2vs'], Wr2vs=W['Wr2vs'], bl2vs=W['bl2vs'].reshape(1, OUT),
                  padsA=padsA, padsB=padsB)
    return dev, shared, meta


def _stack4(w20):
    """[20, X] -> [128, X] with copies at partition offsets 0/32/64/96."""
    out = np.zeros((128, w20.shape[1]), w20.dtype)
    for b in range(4):
        out[32 * b:32 * b + KF] = w20
    return out


def build_bass(shared):
    import concourse.bass as bass
    import concourse.bacc as bacc
    import concourse.mybir as mybir
    import concourse.tile as tile

    padsA, padsB = shared['padsA'], shared['padsB']
    nblkA, nblkB = int(padsA.sum()), int(padsB.sum())
    f32, bf = mybir.dt.float32, mybir.dt.bfloat16

    nc = bacc.Bacc("TRN2", target_bir_lowering=False, debug=False, num_devices=M)
    dt_in = {
        'U_A': ([KF, nblkA * 128], bf), 'U_B': ([KF, nblkB * 128], bf),
        'Vv': ([KF, NV_PAD], bf), 'Vs': ([KF, NS_PAD], bf),
        'rA': ([128, NT_A], f32), 'rB': ([128, NT_B], f32),
        'W1v': ([KF, HID], bf), 'W1s': ([KF, HID], bf),
        'h1v': ([HID, 1], f32), 'h1s': ([HID, 1], f32),
        'Wl2sv': ([HID, OUT], f32), 'Wr2sv': ([HID, OUT], f32),
        'bl2sv': ([1, OUT], f32),
        'Wl2vs': ([HID, OUT], f32), 'Wr2vs': ([HID, OUT], f32),
        'bl2vs': ([1, OUT], f32),
        'ident': ([128, 128], bf), 'ones1': ([1, 128], f32),
    }
    dram = {k: nc.dram_tensor(k, sh, d, kind="ExternalInput")
            for k, (sh, d) in dt_in.items()}
    out_v = nc.dram_tensor("xv2", [NV_PAD, OUT], f32, kind="ExternalOutput")
    out_s = nc.dram_tensor("xs2", [NS_PAD, OUT], f32, kind="ExternalOutput")

    UDMA_BLK = 32    # slot-blocks per U DMA

    with tile.TileContext(nc) as tc:
        with (
            tc.tile_pool(name="const", bufs=1) as cpool,
            tc.tile_pool(name="upool", bufs=3) as upool,
            tc.tile_pool(name="msg", bufs=3) as mpool,
            tc.tile_pool(name="work", bufs=3) as wpool,
            tc.tile_pool(name="big", bufs=1) as bpool,
            tc.tile_pool(name="zp", bufs=2, space="PSUM") as zpool,
            tc.tile_pool(name="accp", bufs=2, space="PSUM") as apool,
            tc.tile_pool(name="tp", bufs=1, space="PSUM") as tpool,
        ):
            C = {}
            for k in ('ident', 'ones1', 'h1v', 'h1s',
                      'Wl2sv', 'Wr2sv', 'bl2sv', 'Wl2vs', 'Wr2vs', 'bl2vs',
                      'rA', 'rB', 'W1v', 'W1s'):
            # fmt: off
                sh, d = dt_in[k]
                t = cpool.tile(sh, d, tag=k)
                nc.sync.dma_start(out=t[:], in_=dram[k][:])
                C[k] = t
            # fmt: on
            VvT = cpool.tile([KF, NV_PAD], bf)
            nc.sync.dma_start(out=VvT[:], in_=dram['Vv'][:])
            VsT = cpool.tile([KF, NS_PAD], bf)
            nc.sync.dma_start(out=VsT[:], in_=dram['Vs'][:])

            x1T_v = bpool.tile([HID, NV_PAD], f32, tag="x1Tv")
            x1T_s = bpool.tile([HID, NS_PAD], f32, tag="x1Ts")
            m2T_v = bpool.tile([HID, NV_PAD], f32, tag="m2Tv")
            m2T_s = bpool.tile([HID, NS_PAD], f32, tag="m2Ts")

            def dense_x1(xT, VT, wkey, hkey, npad):
                for c0 in range(0, npad, 512):
                    w = min(512, npad - c0)
                    ps = zpool.tile([128, 512], f32, space="PSUM", tag="z")
                    nc.tensor.matmul(out=ps[:HID, :w], lhsT=C[wkey][:],
                                     rhs=VT[:, c0:c0 + w], start=True, stop=True)
                    nc.scalar.activation(
                        out=xT[:, c0:c0 + w], in_=ps[:HID, :w],
                        func=mybir.ActivationFunctionType.Relu,
                        bias=C[hkey][:], scale=1.0)

            def edge_pass(which, pads, ntiles, udram, wkey, rkey, m2T):
                boff = 0
                for t in range(ntiles):
                    nblk_t = int(pads[t])
                    acc = apool.tile([128, HID], f32, space="PSUM", tag="acc")
                    done = 0
                    for bb in range(0, nblk_t, UDMA_BLK):
                        bcnt = min(UDMA_BLK, nblk_t - bb)
                        u = upool.tile([KF, UDMA_BLK * 128], bf, tag="u")
                        nc.sync.dma_start(
                            out=u[:, :bcnt * 128],
                            in_=udram[:, (boff + bb) * 128:(boff + bb + bcnt) * 128])
                        for b8 in range(0, bcnt, 8):
                            nb = min(8, bcnt - b8)
                            zps = zpool.tile([128, 512], f32, space="PSUM", tag="z")
                            for i in range(nb):
                                jb = b8 + i
                                nc.tensor.matmul(
                                    out=zps[:, HID * i:HID * (i + 1)],
                                    lhsT=u[:, jb * 128:(jb + 1) * 128],
                                    rhs=C[wkey][:],
                                    start=True, stop=True)
                            msg = mpool.tile([128, 512], bf, tag="m")
                            if (bb + b8) % 16 == 0:
                                nc.scalar.activation(
                                    out=msg[:, :HID * nb], in_=zps[:, :HID * nb],
                                    func=mybir.ActivationFunctionType.Relu)
                            else:
                                nc.vector.tensor_scalar_max(
                                    out=msg[:, :HID * nb], in0=zps[:, :HID * nb],
                                    scalar1=0.0)
                            for i in range(nb):
                                nc.tensor.matmul(
                                    out=acc[:],
                                    lhsT=C['ident'][:],
                                    rhs=msg[:, HID * i:HID * (i + 1)],
                                    start=(done + i == 0),
                                    stop=(done + i == nblk_t - 1),
                                    skip_group_check=True)
                            done += nb
                    boff += nblk_t
                    # mean2 = acc * r ; transpose to feature-major
                    mean2 = wpool.tile([128, HID], f32, tag="mean2")
                    nc.vector.tensor_scalar_mul(
                        out=mean2[:], in0=acc[:], scalar1=C[rkey][:, t:t + 1])
                    tp = tpool.tile([HID, 128], f32, space="PSUM", tag="tp")
                    nc.tensor.transpose(out=tp[:], in_=mean2[:],
                                        identity=identf[:])
                    nc.any.tensor_copy(out=m2T[:, 128 * t:128 * (t + 1)], in_=tp[:])

            def dense_out(m2T, x1T, wl, wr, bl, odram, ntiles):
                for t in range(ntiles):
                    ps = zpool.tile([128, 512], f32, space="PSUM", tag="z")
                    nc.tensor.matmul(out=ps[:, :OUT], lhsT=m2T[:, 128 * t:128 * (t + 1)],
                                     rhs=C[wl][:], start=True, stop=False,
                                     skip_group_check=True)
                    nc.tensor.matmul(out=ps[:, :OUT], lhsT=x1T[:, 128 * t:128 * (t + 1)],
                                     rhs=C[wr][:], start=False, stop=False,
                                     skip_group_check=True)
                    nc.tensor.matmul(out=ps[:, :OUT], lhsT=C['ones1'][:], rhs=C[bl][:],
                                     start=False, stop=True, skip_group_check=True)
                    o = wpool.tile([128, OUT], f32, tag="o")
                    nc.scalar.activation(out=o[:], in_=ps[:, :OUT],
                                         func=mybir.ActivationFunctionType.Relu)
                    nc.sync.dma_start(out=odram[128 * t:128 * (t + 1), :], in_=o[:])

            # transpose identity must be f32 for f32 transpose? use bf ident for
            # reduction; separate f32 identity for transpose:
            identf = cpool.tile([128, 128], f32)
            nc.vector.tensor_copy(out=identf[:], in_=C['ident'][:])

            dense_x1(x1T_v, VvT, 'W1v', 'h1v', NV_PAD)
            dense_x1(x1T_s, VsT, 'W1s', 'h1s', NS_PAD)
            edge_pass('A', padsA, NT_A, dram['U_A'], 'W1s', 'rA', m2T_v)
            edge_pass('B', padsB, NT_B, dram['U_B'], 'W1v', 'rB', m2T_s)
            dense_out(m2T_v, x1T_v, 'Wl2sv', 'Wr2sv', 'bl2sv', out_v, NT_A)
            dense_out(m2T_s, x1T_s, 'Wl2vs', 'Wr2vs', 'bl2vs', out_s, NT_B)

    nc.compile()
    return nc


def _in_maps(dev, shared):
    base = {
        'W1v': np.asarray(shared['W1v']), 'W1s': np.asarray(shared['W1s']),
        'h1v': shared['h1v'], 'h1s': shared['h1s'],
        'Wl2sv': shared['Wl2sv'], 'Wr2sv': shared['Wr2sv'], 'bl2sv': shared['bl2sv'],
        'Wl2vs': shared['Wl2vs'], 'Wr2vs': shared['Wr2vs'], 'bl2vs': shared['bl2vs'],
        'ident': np.eye(128, dtype=bf16), 'ones1': np.ones((1, 128), np.float32),
    }
    maps = []
    for c in range(M):
        m = dict(base)
        m.update(U_A=dev[c]['U_A'], U_B=dev[c]['U_B'],
                 Vv=dev[c]['Vv'], Vs=dev[c]['Vs'],
                 rA=dev[c]['rA'], rB=dev[c]['rB'])
        maps.append(m)
    return maps


_CACHE = {}


def kernel(**inputs):
    import sys
    for p in ("/opt/trn_rl_repo",):
        if p not in sys.path:
            sys.path.insert(0, p)
    from concourse.bass_utils import run_bass_kernel_spmd

    W = {k: np.asarray(v, np.float32) for k, v in inputs.items()
         if k[0] in ('W', 'b')}
    dev, shared, meta = _prep(inputs['x_site'], inputs['x_vendor'],
                              inputs['src'], inputs['dst'], W)
    key = (tuple(shared['padsA'].tolist()), tuple(shared['padsB'].tolist()))
    if key not in _CACHE:
        _CACHE[key] = build_bass(shared)
    nc = _CACHE[key]
    res = run_bass_kernel_spmd(nc, _in_maps(dev, shared), list(range(M)))

    out = np.zeros((NS + NV, OUT), np.float32)
    so, sl = meta['s_owner'], meta['s_local']
    vo, vl = meta['v_owner'], meta['v_local']
    for c in range(M):
        sel = np.flatnonzero(so == c)
        out[sel] = res.results[c]['xs2'][sl[sel]]
        sel = np.flatnonzero(vo == c)
        out[NS + sel] = res.results[c]['xv2'][vl[sel]]
    return out



# revision 3
# speedup vs baseline: 1.4760x; 1.4760x over previous
"""Bipartite 2-layer GraphSAGE encoder on 8 Trainium2 NeuronCores.

Strategy v2 (all edge irregularity resolved on the host into dense,
statically-addressed layouts; device work is a handful of large
streaming matmuls):

  reference:
    xs  = x_site @ Wsi + bsi ; xv = x_vendor @ Wvi + bvi
    xv1 = relu(mean_{dst}(xs[src]) @ Wl1sv + bl1sv + xv @ Wr1sv)
    xs1 = relu(mean_{src}(xv[dst]) @ Wl1vs + bl1vs + xs @ Wr1vs)
    xv2 = relu(mean_{dst}(xs1[src]) @ Wl2sv + bl2sv + xv1 @ Wr2sv)
    xs2 = relu(mean_{src}(xv1[dst]) @ Wl2vs + bl2vs + xs1 @ Wr2vs)

  Folding: every layer-1 activation is an affine map of the raw 19-dim
  concatenated features plus a bias:
    xs1[i] = relu([m9[i] | x_site[i] | 1] @ W1A)    (20 rows, bias row)
  and the layer-2 mean over relu'd messages absorbs the 1/deg scale into
  the per-edge features (relu(r z) = r relu(z), r > 0):
    mean_{e: dst=v} xs1[src_e] = sum_e relu(W1A^T (r_v f_e))
  so per-edge 20-dim columns U = r * [f_e | 1] are built host-side.

  Device per direction:
    MM1:  z[128,512]  = [W1|W1]_blockdiag^T @ U[40,512]   (1024 slots/mm)
    relu: msgT[128,512] (bf16)  <- z   (ScalarE / VectorE alternating)
    MM2:  accT[32,512] += [Wl2;Wl2]^T @ msgT               (sums halves)
    per owner-tile (512 owners): + Wr2^T @ x1T[:,tile], then
    out2T[32,512] = relu(accT + b2)  -> one DMA per direction at the end.

  Edges sharded by owner (vendor for site->vendor, site for the other
  direction), owners globally degree-sorted and dealt round-robin to the
  8 cores; within a core, tiles of 512 owners padded to the tile-max
  degree (pairs of slot-halves). No collectives needed.
"""

import numpy as np
import ml_dtypes

bf16 = ml_dtypes.bfloat16

M = 8
NS, NV, E = 100000, 20000, 3200000
SITE_IN, VENDOR_IN, HID, OUT = 10, 9, 64, 32
NS_LOC, NV_LOC = NS // M, NV // M          # 12500 / 2500
TO = 512                                    # owners per tile
NT_B = (NS_LOC + TO - 1) // TO             # 25 site tiles per core
NT_A = (NV_LOC + TO - 1) // TO             # 5 vendor tiles per core
NS_PAD, NV_PAD = NT_B * TO, NT_A * TO      # 12800 / 2560
KF = 20                                    # 19 features + r (bias) row
BATCH_G = 2                                # MM1/MM2 groups per PSUM tile
UCHUNK_G = 12                              # groups per U DMA


def _owner_maps(deg, n, m):
    order = np.argsort(-deg, kind="stable")
    owner = np.empty(n, np.int32)
    local = np.empty(n, np.int32)
    k = np.arange(n)
    owner[order] = k % m
    local[order] = (k // m).astype(np.int32)
    return owner, local


def _ell(owner, local, n_loc, n_tiles, edge_feat, r_edge, m):
    """Per-core uniform-pad ELL arrays, 512-owner tiles, 2-stacked halves.

    edge_feat: [E, 19] float32 (gathered per-edge features).
    r_edge:    [E] float32 per-edge 1/deg(owner) scale.
    Returns U [m, 2*KF, ncols] bf16 and groups-per-tile G [n_tiles].
    """
    flat = owner.astype(np.int64) * n_loc + local
    counts = np.bincount(flat, minlength=m * n_loc).reshape(m, n_loc)
    G = np.zeros(n_tiles, np.int64)
    for t in range(n_tiles):
        hi = min(TO * (t + 1), n_loc)
        G[t] = max((int(counts[:, TO * t:hi].max()) + 1) // 2, 1)
    tile_off = np.concatenate([[0], np.cumsum(G)]) * TO
    ncols = int(tile_off[-1])
    U = np.zeros((m, 2 * KF, ncols), np.float32)

    order = np.argsort(flat, kind="stable")
    so, sl = owner[order], local[order]
    sf = edge_feat[order] * r_edge[order][:, None]
    sr = r_edge[order]
    starts = np.concatenate([[0], np.cumsum(counts.reshape(-1))])
    pos = np.arange(len(order)) - starts[so.astype(np.int64) * n_loc + sl]
    t_idx = sl // TO
    col = tile_off[t_idx] + (pos // 2) * TO + (sl % TO)
    rb = (pos % 2) * KF
    for f in range(19):
        U[so, rb + f, col] = sf[:, f]
    U[so, rb + 19, col] = sr
    return U.astype(bf16), G


def _prep(x_site, x_vendor, src, dst, W):
    src = np.asarray(src).astype(np.int64)
    dst = np.asarray(dst).astype(np.int64)
    x_site = np.asarray(x_site, np.float32)
    x_vendor = np.asarray(x_vendor, np.float32)

    deg_v = np.bincount(dst, minlength=NV)
    deg_s = np.bincount(src, minlength=NS)
    rv = (1.0 / np.maximum(deg_v, 1)).astype(np.float32)
    rs = (1.0 / np.maximum(deg_s, 1)).astype(np.float32)

    agg10 = np.zeros((NV, SITE_IN), np.float32)
    np.add.at(agg10, dst, x_site[src])
    mean10 = agg10 * rv[:, None]
    agg9 = np.zeros((NS, VENDOR_IN), np.float32)
    np.add.at(agg9, src, x_vendor[dst])
    mean9 = agg9 * rs[:, None]

    v_owner, v_local = _owner_maps(deg_v, NV, M)
    s_owner, s_local = _owner_maps(deg_s, NS, M)

    # direction A: sharded by dst (vendor) owner; messages are xs1[src]
    featA = np.concatenate([mean9[src], x_site[src]], axis=1)
    U_A, G_A = _ell(v_owner[dst], v_local[dst], NV_LOC, NT_A, featA,
                    rv[dst], M)
    # direction B: sharded by src (site) owner; messages are xv1[dst]
    featB = np.concatenate([mean10[dst], x_vendor[dst]], axis=1)
    U_B, G_B = _ell(s_owner[src], s_local[src], NS_LOC, NT_B, featB,
                    rs[src], M)

    # folded layer-1 weights with bias row (KF=20 rows)
    W1A = np.concatenate([
        W['W_vendor_in'] @ W['Wl1vs'], W['W_site_in'] @ W['Wr1vs'],
        (W['b_vendor_in'] @ W['Wl1vs'] + W['bl1vs']
         + W['b_site_in'] @ W['Wr1vs'])[None, :]], axis=0)      # [20,64]
    W1B = np.concatenate([
        W['W_site_in'] @ W['Wl1sv'], W['W_vendor_in'] @ W['Wr1sv'],
        (W['b_site_in'] @ W['Wl1sv'] + W['bl1sv']
         + W['b_vendor_in'] @ W['Wr1sv'])[None, :]], axis=0)    # [20,64]

    def stk40(w):                      # [20,64] -> [40,128] block-diag
        o = np.zeros((2 * KF, 128), np.float32)
        o[:KF, :HID] = w
        o[KF:, HID:] = w
        return o

    # per-core dense node features V [20, n_pad] (19 feats + const-1 row)
    Vv = np.zeros((M, KF, NV_PAD), np.float32)
    Vs = np.zeros((M, KF, NS_PAD), np.float32)
    for c in range(M):
        sel = np.flatnonzero(v_owner == c)
        loc = v_local[sel]
        Vv[c, :SITE_IN, loc] = mean10[sel]
        Vv[c, SITE_IN:19, loc] = x_vendor[sel]
        Vv[c, 19, loc] = 1.0
        sel = np.flatnonzero(s_owner == c)
        loc = s_local[sel]
        Vs[c, :VENDOR_IN, loc] = mean9[sel]
        Vs[c, VENDOR_IN:19, loc] = x_site[sel]
        Vs[c, 19, loc] = 1.0

    meta = dict(v_owner=v_owner, v_local=v_local,
                s_owner=s_owner, s_local=s_local)
    dev = []
    for c in range(M):
        dev.append(dict(U_A=np.ascontiguousarray(U_A[c]),
                        U_B=np.ascontiguousarray(U_B[c]),
                        Vv=Vv[c].astype(bf16), Vs=Vs[c].astype(bf16)))
    shared = dict(
        W1A=W1A.astype(bf16), W1B=W1B.astype(bf16),
        W1Astk=stk40(W1A).astype(bf16), W1Bstk=stk40(W1B).astype(bf16),
        Wl2Astk=np.concatenate([W['Wl2sv'], W['Wl2sv']], 0).astype(bf16),
        Wl2Bstk=np.concatenate([W['Wl2vs'], W['Wl2vs']], 0).astype(bf16),
        Wr2A=W['Wr2sv'].astype(bf16), Wr2B=W['Wr2vs'].astype(bf16),
        b2A=W['bl2sv'].reshape(OUT, 1).astype(np.float32),
        b2B=W['bl2vs'].reshape(OUT, 1).astype(np.float32),
        G_A=G_A, G_B=G_B)
    return dev, shared, meta


def build_bass(shared):
    import concourse.bass as bass
    import concourse.bacc as bacc
    import concourse.mybir as mybir
    import concourse.tile as tile

    G_A, G_B = shared['G_A'], shared['G_B']
    ncolsA = int(G_A.sum()) * TO
    ncolsB = int(G_B.sum()) * TO
    f32, bf = mybir.dt.float32, mybir.dt.bfloat16
    Relu = mybir.ActivationFunctionType.Relu

    nc = bacc.Bacc("TRN2", target_bir_lowering=False, debug=False,
                   num_devices=M)
    dt_in = {
        'U_A': ([2 * KF, ncolsA], bf), 'U_B': ([2 * KF, ncolsB], bf),
        'Vv': ([KF, NV_PAD], bf), 'Vs': ([KF, NS_PAD], bf),
        'W1A': ([KF, HID], bf), 'W1B': ([KF, HID], bf),
        'W1Astk': ([2 * KF, 128], bf), 'W1Bstk': ([2 * KF, 128], bf),
        'Wl2Astk': ([128, OUT], bf), 'Wl2Bstk': ([128, OUT], bf),
        'Wr2A': ([HID, OUT], bf), 'Wr2B': ([HID, OUT], bf),
        'b2A': ([OUT, 1], f32), 'b2B': ([OUT, 1], f32),
    }
    dram = {k: nc.dram_tensor(k, sh, d, kind="ExternalInput")
            for k, (sh, d) in dt_in.items()}
    out_v = nc.dram_tensor("xv2", [OUT, NV_PAD], f32, kind="ExternalOutput")
    out_s = nc.dram_tensor("xs2", [OUT, NS_PAD], f32, kind="ExternalOutput")

    with tile.TileContext(nc) as tc:
        with (
            tc.tile_pool(name="const", bufs=1) as cpool,
            tc.tile_pool(name="upool", bufs=3) as upool,
            tc.tile_pool(name="msg", bufs=4) as mpool,
            tc.tile_pool(name="big", bufs=1) as bpool,
            tc.tile_pool(name="zp", bufs=3, space="PSUM") as zpool,
            tc.tile_pool(name="accp", bufs=2, space="PSUM") as apool,
        ):
            C = {}
            for k in dt_in:
                if k.startswith('U_'):
                    continue
                sh, d = dt_in[k]
                t = cpool.tile(sh, d, tag=k)
                nc.sync.dma_start(out=t[:], in_=dram[k][:])
                C[k] = t

            x1T_v = bpool.tile([HID, NV_PAD], bf, tag="x1Tv")
            x1T_s = bpool.tile([HID, NS_PAD], bf, tag="x1Ts")
            oT_v = bpool.tile([OUT, NV_PAD], f32, tag="oTv")
            oT_s = bpool.tile([OUT, NS_PAD], f32, tag="oTs")

            def dense_x1(xT, vkey, wkey, npad):
                for i, c0 in enumerate(range(0, npad, TO)):
                    ps = zpool.tile([128, BATCH_G * TO], f32, space="PSUM",
                                    tag="z")
                    nc.tensor.matmul(out=ps[:HID, :TO], lhsT=C[wkey][:],
                                     rhs=C[vkey][:, c0:c0 + TO],
                                     start=True, stop=True)
                    if i % 2 == 0:
                        nc.scalar.activation(out=xT[:, c0:c0 + TO],
                                             in_=ps[:HID, :TO], func=Relu)
                    else:
                        nc.vector.tensor_scalar_max(
                            out=xT[:, c0:c0 + TO], in0=ps[:HID, :TO],
                            scalar1=0.0)

            def edge_pass(G, ntiles, udram, w1key, wl2key, wr2key, bkey,
                          x1T, oT, relu_phase):
                # build batch list: (tile, col_off, ngroups, first, last)
                batches = []
                off = 0
                for t in range(ntiles):
                    g = int(G[t])
                    done = 0
                    while done < g:
                        ng = min(BATCH_G, g - done)
                        batches.append((t, off + done * TO, ng,
                                        done == 0, done + ng == g))
                        done += ng
                    off += g * TO
                # U chunk DMAs: per tile, up to UCHUNK_G groups each
                uchunks = []          # (col_off, ncols)
                off = 0
                for t in range(ntiles):
                    g = int(G[t])
                    done = 0
                    while done < g:
                        ng = min(UCHUNK_G, g - done)
                        uchunks.append((off + done * TO, ng * TO))
                        done += ng
                    off += g * TO
                chunk_tiles = {}
                ci = 0

                def ensure_chunk(col_off, width):
                    nonlocal ci
                    # find chunk containing [col_off, col_off+width)
                    while not (uchunks[ci][0] <= col_off
                               and col_off + width
                               <= uchunks[ci][0] + uchunks[ci][1]):
                        ci += 1
                    if ci not in chunk_tiles:
                        co, cw = uchunks[ci]
                        u = upool.tile([2 * KF, UCHUNK_G * TO], bf, tag="u")
                        nc.sync.dma_start(out=u[:, :cw],
                                          in_=udram[:, co:co + cw])
                        chunk_tiles[ci] = (u, co)
                    return chunk_tiles[ci]

                D = 2                 # software pipeline depth (batches)
                pend = []             # (ps, msg, batch_info)
                acc = {}              # tile -> psum acc tile

                def do_mm2(ps, msg, binfo):
                    t, coff, ng, first, last = binfo
                    if first:
                        acc[t] = apool.tile([OUT, TO], f32, space="PSUM",
                                            name="acc", tag="acc")
                    a = acc[t]
                    for j in range(ng):
                        nc.tensor.matmul(
                            out=a[:], lhsT=C[wl2key][:],
                            rhs=msg[:, j * TO:(j + 1) * TO],
                            start=(first and j == 0), stop=False,
                            skip_group_check=True)
                    if last:
                        nc.tensor.matmul(
                            out=a[:], lhsT=C[wr2key][:],
                            rhs=x1T[:, t * TO:(t + 1) * TO],
                            start=False, stop=True, skip_group_check=True)
                        nc.scalar.activation(
                            out=oT[:, t * TO:(t + 1) * TO], in_=a[:],
                            func=Relu, bias=C[bkey][:], scale=1.0)
                        del acc[t]

                for i, binfo in enumerate(batches):
                    t, coff, ng, first, last = binfo
                    u, co = ensure_chunk(coff, ng * TO)
                    ps = zpool.tile([128, BATCH_G * TO], f32, space="PSUM",
                                    tag="z")
                    for j in range(ng):
                        nc.tensor.matmul(
                            out=ps[:, j * TO:(j + 1) * TO],
                            lhsT=C[w1key][:],
                            rhs=u[:, coff - co + j * TO:
                                  coff - co + (j + 1) * TO],
                            start=True, stop=True)
                    msg = mpool.tile([128, BATCH_G * TO], bf, tag="m")
                    if (i + relu_phase) % 2 == 0:
                        nc.scalar.activation(out=msg[:, :ng * TO],
                                             in_=ps[:, :ng * TO], func=Relu)
                    else:
                        nc.vector.tensor_scalar_max(
                            out=msg[:, :ng * TO], in0=ps[:, :ng * TO],
                            scalar1=0.0)
                    pend.append((ps, msg, binfo))
                    if len(pend) > D:
                        do_mm2(*pend.pop(0))
                while pend:
                    do_mm2(*pend.pop(0))

            dense_x1(x1T_v, 'Vv', 'W1B', NV_PAD)
            dense_x1(x1T_s, 'Vs', 'W1A', NS_PAD)
            edge_pass(G_A, NT_A, dram['U_A'], 'W1Astk', 'Wl2Astk', 'Wr2A',
                      'b2A', x1T_v, oT_v, 0)
            edge_pass(G_B, NT_B, dram['U_B'], 'W1Bstk', 'Wl2Bstk', 'Wr2B',
                      'b2B', x1T_s, oT_s, 1)
            nc.sync.dma_start(out=out_v[:], in_=oT_v[:])
            nc.sync.dma_start(out=out_s[:], in_=oT_s[:])

    nc.compile()
    return nc


def _in_maps(dev, shared):
    base = {k: np.asarray(shared[k]) for k in
            ('W1A', 'W1B', 'W1Astk', 'W1Bstk', 'Wl2Astk', 'Wl2Bstk',
             'Wr2A', 'Wr2B', 'b2A', 'b2B')}
    maps = []
    for c in range(M):
        m = dict(base)
        m.update(U_A=dev[c]['U_A'], U_B=dev[c]['U_B'],
                 Vv=dev[c]['Vv'], Vs=dev[c]['Vs'])
        maps.append(m)
    return maps


_CACHE = {}


def kernel(**inputs):
    import sys
    for p in ("/opt/trn_rl_repo",):
        if p not in sys.path:
            sys.path.insert(0, p)
    from concourse.bass_utils import run_bass_kernel_spmd

    W = {k: np.asarray(v, np.float32) for k, v in inputs.items()
         if k[0] in ('W', 'b')}
    dev, shared, meta = _prep(inputs['x_site'], inputs['x_vendor'],
                              inputs['src'], inputs['dst'], W)
    key = (tuple(shared['G_A'].tolist()), tuple(shared['G_B'].tolist()))
    if key not in _CACHE:
        _CACHE[key] = build_bass(shared)
    nc = _CACHE[key]
    res = run_bass_kernel_spmd(nc, _in_maps(dev, shared), list(range(M)))

    out = np.zeros((NS + NV, OUT), np.float32)
    so, sl = meta['s_owner'], meta['s_local']
    vo, vl = meta['v_owner'], meta['v_local']
    for c in range(M):
        xs2 = np.ascontiguousarray(res.results[c]['xs2'].T)
        xv2 = np.ascontiguousarray(res.results[c]['xv2'].T)
        sel = np.flatnonzero(so == c)
        out[sel] = xs2[sl[sel]]
        sel = np.flatnonzero(vo == c)
        out[NS + sel] = xv2[vl[sel]]
    return out


# revision 5
# speedup vs baseline: 2.4040x; 1.6287x over previous
"""Bipartite 2-layer GraphSAGE encoder on 8 Trainium2 NeuronCores.

Strategy v3. All edge irregularity is resolved on the host into dense,
statically-addressed layouts; the device runs a pure edge pipeline of
large streaming matmuls; the cheap dense node-term and final relu run
on the host in f32.

  reference:
    xs  = x_site @ Wsi + bsi ; xv = x_vendor @ Wvi + bvi
    xv1 = relu(mean_{dst}(xs[src]) @ Wl1sv + bl1sv + xv @ Wr1sv)
    xs1 = relu(mean_{src}(xv[dst]) @ Wl1vs + bl1vs + xs @ Wr1vs)
    xv2 = relu(mean_{dst}(xs1[src]) @ Wl2sv + bl2sv + xv1 @ Wr2sv)
    xs2 = relu(mean_{src}(xv1[dst]) @ Wl2vs + bl2vs + xs1 @ Wr2vs)

  Every layer-1 activation is an affine map of raw 19-dim concatenated
  features: xs1[i] = relu([m9[i] | x_site[i] | 1] @ W1A) (20 rows incl
  bias row). The device computes, per destination owner o,
    S[:, o] = sum_{e -> o} Wl2^T relu(W1^T u_e)     (u_e: 20-dim, fp8)
  via two 512-column streaming matmuls per 1024 edge slots:
    MM1:  z[128,512]  = [W1|W1]_blockdiag^T @ U[40,512]
    relu: msgT[128,1024] (bf16) <- z  (ScalarE/VectorE alternating)
    MM2:  accT[32,512] += [Wl2;Wl2]^T @ msgT       (sums slot halves)
  Stationaries swap only once per 12-group mega-batch so the PE streams
  at its 1-col/cycle limit. The host finishes with (f32)
    out[o] = relu(S[:, o]/deg_o + x1[o] @ Wr2 + b2).

  Edges sharded by owner (vendor for site->vendor, site for the other
  direction), owners globally degree-sorted and dealt round-robin to the
  8 cores; within a core, tiles of 512 owners padded to the tile-max
  degree (pairs of slot-halves). Pad slots are all-zero (including the
  ones-row) so they contribute nothing. No collectives needed.
"""

import numpy as np
import ml_dtypes

bf16 = ml_dtypes.bfloat16
fp8 = ml_dtypes.float8_e4m3

M = 8
NS, NV, E = 100000, 20000, 3200000
SITE_IN, VENDOR_IN, HID, OUT = 10, 9, 64, 32
NS_LOC, NV_LOC = NS // M, NV // M          # 12500 / 2500
TO = 512                                    # owners per tile
NT_B = (NS_LOC + TO - 1) // TO             # 25 site tiles per core
NT_A = (NV_LOC + TO - 1) // TO             # 5 vendor tiles per core
NS_PAD, NV_PAD = NT_B * TO, NT_A * TO      # 12800 / 2560
KF = 20                                    # 19 features + ones row
CHUNK_G = 12                               # groups per mega-batch / U DMA


def _owner_maps(deg, n, m):
    order = np.argsort(-deg, kind="stable")
    owner = np.empty(n, np.int32)
    local = np.empty(n, np.int32)
    k = np.arange(n)
    owner[order] = k % m
    local[order] = (k // m).astype(np.int32)
    return owner, local


def _ell(owner, local, n_loc, n_tiles, edge_feat, m):
    """Per-core uniform-pad ELL arrays, 512-owner tiles, 2-stacked halves.

    edge_feat: [E, 19] float32 (gathered, unscaled per-edge features).
    Returns U [m, 2*KF, ncols] fp8 and groups-per-tile G [n_tiles].
    """
    flat = owner.astype(np.int64) * n_loc + local
    counts = np.bincount(flat, minlength=m * n_loc).reshape(m, n_loc)
    G = np.zeros(n_tiles, np.int64)
    for t in range(n_tiles):
        hi = min(TO * (t + 1), n_loc)
        G[t] = max((int(counts[:, TO * t:hi].max()) + 1) // 2, 1)
    tile_off = np.concatenate([[0], np.cumsum(G)]) * TO
    ncols = int(tile_off[-1])
    U = np.zeros((m, 2 * KF, ncols), np.float32)

    order = np.argsort(flat, kind="stable")
    so, sl = owner[order], local[order]
    sf = edge_feat[order]
    starts = np.concatenate([[0], np.cumsum(counts.reshape(-1))])
    pos = np.arange(len(order)) - starts[so.astype(np.int64) * n_loc + sl]
    t_idx = sl // TO
    col = tile_off[t_idx] + (pos // 2) * TO + (sl % TO)
    rb = (pos % 2) * KF
    for f in range(19):
        U[so, rb + f, col] = sf[:, f]
    U[so, rb + 19, col] = 1.0
    return U.astype(fp8), G


def _prep(x_site, x_vendor, src, dst, W):
    src = np.asarray(src).astype(np.int64)
    dst = np.asarray(dst).astype(np.int64)
    x_site = np.asarray(x_site, np.float32)
    x_vendor = np.asarray(x_vendor, np.float32)

    deg_v = np.bincount(dst, minlength=NV)
    deg_s = np.bincount(src, minlength=NS)
    rv = (1.0 / np.maximum(deg_v, 1)).astype(np.float32)
    rs = (1.0 / np.maximum(deg_s, 1)).astype(np.float32)

    agg10 = np.zeros((NV, SITE_IN), np.float32)
    np.add.at(agg10, dst, x_site[src])
    mean10 = agg10 * rv[:, None]
    agg9 = np.zeros((NS, VENDOR_IN), np.float32)
    np.add.at(agg9, src, x_vendor[dst])
    mean9 = agg9 * rs[:, None]

    v_owner, v_local = _owner_maps(deg_v, NV, M)
    s_owner, s_local = _owner_maps(deg_s, NS, M)

    # direction A: sharded by dst (vendor) owner; messages are xs1[src]
    featA = np.concatenate([mean9[src], x_site[src]], axis=1)
    U_A, G_A = _ell(v_owner[dst], v_local[dst], NV_LOC, NT_A, featA, M)
    # direction B: sharded by src (site) owner; messages are xv1[dst]
    featB = np.concatenate([mean10[dst], x_vendor[dst]], axis=1)
    U_B, G_B = _ell(s_owner[src], s_local[src], NS_LOC, NT_B, featB, M)

    # folded layer-1 weights with bias row (KF=20 rows)
    W1A = np.concatenate([
        W['W_vendor_in'] @ W['Wl1vs'], W['W_site_in'] @ W['Wr1vs'],
        (W['b_vendor_in'] @ W['Wl1vs'] + W['bl1vs']
         + W['b_site_in'] @ W['Wr1vs'])[None, :]], axis=0)      # [20,64]
    W1B = np.concatenate([
        W['W_site_in'] @ W['Wl1sv'], W['W_vendor_in'] @ W['Wr1sv'],
        (W['b_site_in'] @ W['Wl1sv'] + W['bl1sv']
         + W['b_vendor_in'] @ W['Wr1sv'])[None, :]], axis=0)    # [20,64]

    def stk40(w):                      # [20,64] -> [40,128] block-diag
        o = np.zeros((2 * KF, 128), np.float32)
        o[:KF, :HID] = w
        o[KF:, HID:] = w
        return o

    # host-side dense terms (all f32, matching the reference exactly)
    x1_site = np.maximum(
        np.concatenate([mean9, x_site], 1) @ W1A[:19] + W1A[19], 0)
    x1_vendor = np.maximum(
        np.concatenate([mean10, x_vendor], 1) @ W1B[:19] + W1B[19], 0)
    T_s = x1_site @ W['Wr2vs'] + W['bl2vs']       # own-term for sites
    T_v = x1_vendor @ W['Wr2sv'] + W['bl2sv']     # own-term for vendors

    meta = dict(v_owner=v_owner, v_local=v_local,
                s_owner=s_owner, s_local=s_local,
                T_s=T_s, T_v=T_v, rv=rv, rs=rs)
    dev = [dict(U_A=np.ascontiguousarray(U_A[c]),
                U_B=np.ascontiguousarray(U_B[c])) for c in range(M)]
    shared = dict(
        W1Astk=stk40(W1A).astype(bf16), W1Bstk=stk40(W1B).astype(bf16),
        Wl2Astk=np.concatenate([W['Wl2sv'], W['Wl2sv']], 0).astype(bf16),
        Wl2Bstk=np.concatenate([W['Wl2vs'], W['Wl2vs']], 0).astype(bf16),
        G_A=G_A, G_B=G_B)
    return dev, shared, meta


def build_bass(shared):
    import concourse.bass as bass
    import concourse.bacc as bacc
    import concourse.mybir as mybir
    import concourse.tile as tile

    G_A, G_B = shared['G_A'], shared['G_B']
    ncolsA = int(G_A.sum()) * TO
    ncolsB = int(G_B.sum()) * TO
    f32, bf = mybir.dt.float32, mybir.dt.bfloat16
    f8 = mybir.dt.float8e4
    Relu = mybir.ActivationFunctionType.Relu
    Copy = mybir.ActivationFunctionType.Copy

    nc = bacc.Bacc("TRN2", target_bir_lowering=False, debug=False,
                   num_devices=M)
    dt_in = {
        'U_A': ([2 * KF, ncolsA], f8), 'U_B': ([2 * KF, ncolsB], f8),
        'W1Astk': ([2 * KF, 128], bf), 'W1Bstk': ([2 * KF, 128], bf),
        'Wl2Astk': ([128, OUT], bf), 'Wl2Bstk': ([128, OUT], bf),
    }
    dram = {k: nc.dram_tensor(k, sh, d, kind="ExternalInput")
            for k, (sh, d) in dt_in.items()}
    out_v = nc.dram_tensor("xv2", [OUT, NV_PAD], f32, kind="ExternalOutput")
    out_s = nc.dram_tensor("xs2", [OUT, NS_PAD], f32, kind="ExternalOutput")

    with tile.TileContext(nc) as tc:
        with (
            tc.tile_pool(name="const", bufs=1) as cpool,
            tc.tile_pool(name="upool", bufs=3) as upool,
            tc.tile_pool(name="msg", bufs=8) as mpool,
            tc.tile_pool(name="big", bufs=1) as bpool,
            tc.tile_pool(name="zp", bufs=3, space="PSUM") as zpool,
            tc.tile_pool(name="accp", bufs=2, space="PSUM") as apool,
        ):
            C = {}
            for k in dt_in:
                if k.startswith('U_'):
                    continue
                sh, d = dt_in[k]
                t = cpool.tile(sh, d, tag=k)
                nc.sync.dma_start(out=t[:], in_=dram[k][:])
                C[k] = t

            oT_v = bpool.tile([OUT, NV_PAD], f32, tag="oTv")
            oT_s = bpool.tile([OUT, NS_PAD], f32, tag="oTs")

            relu_i = [0]
            copy_i = [0]

            def edge_pass(G, ntiles, udram, w1key, wl2key, oT):
                off = 0
                for t in range(ntiles):
                    g = int(G[t])
                    acc = apool.tile([OUT, TO], f32, space="PSUM",
                                     name="acc", tag="acc")
                    done = 0
                    first = True
                    while done < g:
                        ng = min(CHUNK_G, g - done)
                        coff = off + done * TO
                        u = upool.tile([2 * KF, CHUNK_G * TO], f8, tag="u")
                        nc.sync.dma_start(out=u[:, :ng * TO],
                                          in_=udram[:, coff:coff + ng * TO])
                        msgs = []
                        for z0 in range(0, ng, 2):
                            nz = min(2, ng - z0)
                            ps = zpool.tile([128, 2 * TO], f32,
                                            space="PSUM", name="z", tag="z")
                            for j in range(nz):
                                nc.tensor.matmul(
                                    out=ps[:, j * TO:(j + 1) * TO],
                                    lhsT=C[w1key][:],
                                    rhs=u[:, (z0 + j) * TO:(z0 + j + 1) * TO],
                                    start=True, stop=True)
                            msg = mpool.tile([128, 2 * TO], bf, tag="m")
                            if relu_i[0] % 2 == 0:
                                nc.scalar.activation(
                                    out=msg[:, :nz * TO],
                                    in_=ps[:, :nz * TO], func=Relu)
                            else:
                                nc.vector.tensor_scalar_max(
                                    out=msg[:, :nz * TO],
                                    in0=ps[:, :nz * TO], scalar1=0.0)
                            relu_i[0] += 1
                            msgs.append((msg, nz))
                        nmm2 = 0
                        for msg, nz in msgs:
                            for j in range(nz):
                                nmm2 += 1
                                nc.tensor.matmul(
                                    out=acc[:], lhsT=C[wl2key][:],
                                    rhs=msg[:, j * TO:(j + 1) * TO],
                                    start=first,
                                    stop=(done + nmm2 == g),
                                    skip_group_check=True)
                                first = False
                        done += ng
                    if copy_i[0] % 2 == 0:
                        nc.vector.tensor_copy(
                            out=oT[:, t * TO:(t + 1) * TO], in_=acc[:])
                    else:
                        nc.scalar.activation(
                            out=oT[:, t * TO:(t + 1) * TO], in_=acc[:],
                            func=Copy)
                    copy_i[0] += 1
                    off += g * TO

            edge_pass(G_A, NT_A, dram['U_A'], 'W1Astk', 'Wl2Astk', oT_v)
            edge_pass(G_B, NT_B, dram['U_B'], 'W1Bstk', 'Wl2Bstk', oT_s)
            nc.sync.dma_start(out=out_v[:], in_=oT_v[:])
            nc.sync.dma_start(out=out_s[:], in_=oT_s[:])

    nc.compile()
    return nc


def _in_maps(dev, shared):
    base = {k: np.asarray(shared[k]) for k in
            ('W1Astk', 'W1Bstk', 'Wl2Astk', 'Wl2Bstk')}
    maps = []
    for c in range(M):
        m = dict(base)
        m.update(U_A=dev[c]['U_A'], U_B=dev[c]['U_B'])
        maps.append(m)
    return maps


_CACHE = {}


def kernel(**inputs):
    import sys
    for p in ("/opt/trn_rl_repo",):
        if p not in sys.path:
            sys.path.insert(0, p)
    from concourse.bass_utils import run_bass_kernel_spmd

    W = {k: np.asarray(v, np.float32) for k, v in inputs.items()
         if k[0] in ('W', 'b')}
    dev, shared, meta = _prep(inputs['x_site'], inputs['x_vendor'],
                              inputs['src'], inputs['dst'], W)
    key = (tuple(shared['G_A'].tolist()), tuple(shared['G_B'].tolist()))
    if key not in _CACHE:
        _CACHE[key] = build_bass(shared)
    nc = _CACHE[key]
    res = run_bass_kernel_spmd(nc, _in_maps(dev, shared), list(range(M)))

    out = np.zeros((NS + NV, OUT), np.float32)
    so, sl = meta['s_owner'], meta['s_local']
    vo, vl = meta['v_owner'], meta['v_local']
    for c in range(M):
        S_s = np.ascontiguousarray(res.results[c]['xs2'].T)   # [NS_PAD,32]
        S_v = np.ascontiguousarray(res.results[c]['xv2'].T)
        sel = np.flatnonzero(so == c)
        out[sel] = np.maximum(
            S_s[sl[sel]] * meta['rs'][sel][:, None] + meta['T_s'][sel], 0)
        sel = np.flatnonzero(vo == c)
        out[NS + sel] = np.maximum(
            S_v[vl[sel]] * meta['rv'][sel][:, None] + meta['T_v'][sel], 0)
    return out


# revision 9
# speedup vs baseline: 2.6611x; 1.1069x over previous
"""Bipartite 2-layer GraphSAGE encoder on 8 Trainium2 NeuronCores.

Strategy v3. All edge irregularity is resolved on the host into dense,
statically-addressed layouts; the device runs a pure edge pipeline of
large streaming matmuls; the cheap dense node-term and final relu run
on the host in f32.

  reference:
    xs  = x_site @ Wsi + bsi ; xv = x_vendor @ Wvi + bvi
    xv1 = relu(mean_{dst}(xs[src]) @ Wl1sv + bl1sv + xv @ Wr1sv)
    xs1 = relu(mean_{src}(xv[dst]) @ Wl1vs + bl1vs + xs @ Wr1vs)
    xv2 = relu(mean_{dst}(xs1[src]) @ Wl2sv + bl2sv + xv1 @ Wr2sv)
    xs2 = relu(mean_{src}(xv1[dst]) @ Wl2vs + bl2vs + xs1 @ Wr2vs)

  Every layer-1 activation is an affine map of raw 19-dim concatenated
  features: xs1[i] = relu([m9[i] | x_site[i] | 1] @ W1A) (20 rows incl
  bias row). The device computes, per destination owner o,
    S[:, o] = sum_{e -> o} Wl2^T relu(W1^T u_e)     (u_e: 20-dim, fp8)
  via two 512-column streaming matmuls per 1024 edge slots:
    MM1:  z[128,512]  = [W1|W1]_blockdiag^T @ U[40,512]
    relu: msgT[128,1024] (bf16) <- z  (ScalarE/VectorE alternating)
    MM2:  accT[32,512] += [Wl2;Wl2]^T @ msgT       (sums slot halves)
  Stationaries swap only once per 12-group mega-batch so the PE streams
  at its 1-col/cycle limit. The host finishes with (f32)
    out[o] = relu(S[:, o]/deg_o + x1[o] @ Wr2 + b2).

  Edges sharded by owner (vendor for site->vendor, site for the other
  direction), owners globally degree-sorted and dealt round-robin to the
  8 cores; within a core, tiles of 512 owners padded to the tile-max
  degree (pairs of slot-halves). Pad slots are all-zero (including the
  ones-row) so they contribute nothing. No collectives needed.
"""

import numpy as np
import ml_dtypes

bf16 = ml_dtypes.bfloat16
fp8 = ml_dtypes.float8_e4m3

M = 8
NS, NV, E = 100000, 20000, 3200000
SITE_IN, VENDOR_IN, HID, OUT = 10, 9, 64, 32
NS_LOC, NV_LOC = NS // M, NV // M          # 12500 / 2500
TO = 512                                    # owners per tile
NT_B = (NS_LOC + TO - 1) // TO             # 25 site tiles per core
NT_A = (NV_LOC + TO - 1) // TO             # 5 vendor tiles per core
NS_PAD, NV_PAD = NT_B * TO, NT_A * TO      # 12800 / 2560
KF = 20                                    # 19 features + ones row
CHUNK_G = 24                               # groups per U DMA chunk
BURST_G = 6                                # groups per PE burst (3 z-pairs)


def _owner_maps(deg, n, m):
    order = np.argsort(-deg, kind="stable")
    owner = np.empty(n, np.int32)
    local = np.empty(n, np.int32)
    k = np.arange(n)
    owner[order] = k % m
    local[order] = (k // m).astype(np.int32)
    return owner, local


def _ell(owner, local, n_loc, n_tiles, edge_feat, m):
    """Per-core uniform-pad ELL arrays, 512-owner tiles, 2-stacked halves.

    edge_feat: [E, 19] float32 (gathered, unscaled per-edge features).
    Returns U [m, 2*KF, ncols] fp8 and groups-per-tile G [n_tiles].
    """
    flat = owner.astype(np.int64) * n_loc + local
    counts = np.bincount(flat, minlength=m * n_loc).reshape(m, n_loc)
    G = np.zeros(n_tiles, np.int64)
    for t in range(n_tiles):
        hi = min(TO * (t + 1), n_loc)
        G[t] = max((int(counts[:, TO * t:hi].max()) + 1) // 2, 1)
    tile_off = np.concatenate([[0], np.cumsum(G)]) * TO
    ncols = int(tile_off[-1])
    U = np.zeros((m, 2 * KF, ncols), np.float32)

    order = np.argsort(flat, kind="stable")
    so, sl = owner[order], local[order]
    sf = edge_feat[order]
    starts = np.concatenate([[0], np.cumsum(counts.reshape(-1))])
    pos = np.arange(len(order)) - starts[so.astype(np.int64) * n_loc + sl]
    t_idx = sl // TO
    col = tile_off[t_idx] + (pos // 2) * TO + (sl % TO)
    rb = (pos % 2) * KF
    for f in range(19):
        U[so, rb + f, col] = sf[:, f]
    U[so, rb + 19, col] = 1.0
    return U.astype(fp8), G


def _prep(x_site, x_vendor, src, dst, W):
    src = np.asarray(src).astype(np.int64)
    dst = np.asarray(dst).astype(np.int64)
    x_site = np.asarray(x_site, np.float32)
    x_vendor = np.asarray(x_vendor, np.float32)

    deg_v = np.bincount(dst, minlength=NV)
    deg_s = np.bincount(src, minlength=NS)
    rv = (1.0 / np.maximum(deg_v, 1)).astype(np.float32)
    rs = (1.0 / np.maximum(deg_s, 1)).astype(np.float32)

    agg10 = np.zeros((NV, SITE_IN), np.float32)
    np.add.at(agg10, dst, x_site[src])
    mean10 = agg10 * rv[:, None]
    agg9 = np.zeros((NS, VENDOR_IN), np.float32)
    np.add.at(agg9, src, x_vendor[dst])
    mean9 = agg9 * rs[:, None]

    v_owner, v_local = _owner_maps(deg_v, NV, M)
    s_owner, s_local = _owner_maps(deg_s, NS, M)

    # direction A: sharded by dst (vendor) owner; messages are xs1[src]
    featA = np.concatenate([mean9[src], x_site[src]], axis=1)
    U_A, G_A = _ell(v_owner[dst], v_local[dst], NV_LOC, NT_A, featA, M)
    # direction B: sharded by src (site) owner; messages are xv1[dst]
    featB = np.concatenate([mean10[dst], x_vendor[dst]], axis=1)
    U_B, G_B = _ell(s_owner[src], s_local[src], NS_LOC, NT_B, featB, M)

    # folded layer-1 weights with bias row (KF=20 rows)
    W1A = np.concatenate([
        W['W_vendor_in'] @ W['Wl1vs'], W['W_site_in'] @ W['Wr1vs'],
        (W['b_vendor_in'] @ W['Wl1vs'] + W['bl1vs']
         + W['b_site_in'] @ W['Wr1vs'])[None, :]], axis=0)      # [20,64]
    W1B = np.concatenate([
        W['W_site_in'] @ W['Wl1sv'], W['W_vendor_in'] @ W['Wr1sv'],
        (W['b_site_in'] @ W['Wl1sv'] + W['bl1sv']
         + W['b_vendor_in'] @ W['Wr1sv'])[None, :]], axis=0)    # [20,64]

    def stk40(w):                      # [20,64] -> [40,128] block-diag
        o = np.zeros((2 * KF, 128), np.float32)
        o[:KF, :HID] = w
        o[KF:, HID:] = w
        return o

    # host-side dense terms (all f32, matching the reference exactly)
    x1_site = np.maximum(
        np.concatenate([mean9, x_site], 1) @ W1A[:19] + W1A[19], 0)
    x1_vendor = np.maximum(
        np.concatenate([mean10, x_vendor], 1) @ W1B[:19] + W1B[19], 0)
    T_s = x1_site @ W['Wr2vs'] + W['bl2vs']       # own-term for sites
    T_v = x1_vendor @ W['Wr2sv'] + W['bl2sv']     # own-term for vendors

    meta = dict(v_owner=v_owner, v_local=v_local,
                s_owner=s_owner, s_local=s_local,
                T_s=T_s, T_v=T_v, rv=rv, rs=rs)
    dev = [dict(U_A=np.ascontiguousarray(U_A[c]),
                U_B=np.ascontiguousarray(U_B[c])) for c in range(M)]
    shared = dict(
        W1Astk=stk40(W1A).astype(bf16), W1Bstk=stk40(W1B).astype(bf16),
        Wl2Astk=np.concatenate([W['Wl2sv'], W['Wl2sv']], 0).astype(bf16),
        Wl2Bstk=np.concatenate([W['Wl2vs'], W['Wl2vs']], 0).astype(bf16),
        G_A=G_A, G_B=G_B)
    return dev, shared, meta


def build_bass(shared):
    import concourse.bass as bass
    import concourse.bacc as bacc
    import concourse.mybir as mybir
    import concourse.tile as tile

    G_A, G_B = shared['G_A'], shared['G_B']
    ncolsA = int(G_A.sum()) * TO
    ncolsB = int(G_B.sum()) * TO
    f32, bf = mybir.dt.float32, mybir.dt.bfloat16
    f8 = mybir.dt.float8e4
    Relu = mybir.ActivationFunctionType.Relu
    Copy = mybir.ActivationFunctionType.Copy

    nc = bacc.Bacc("TRN2", target_bir_lowering=False, debug=False,
                   num_devices=M)
    dt_in = {
        'U_A': ([2 * KF, ncolsA], f8), 'U_B': ([2 * KF, ncolsB], f8),
        'W1Astk': ([2 * KF, 128], bf), 'W1Bstk': ([2 * KF, 128], bf),
        'Wl2Astk': ([128, OUT], bf), 'Wl2Bstk': ([128, OUT], bf),
    }
    dram = {k: nc.dram_tensor(k, sh, d, kind="ExternalInput")
            for k, (sh, d) in dt_in.items()}
    out_v = nc.dram_tensor("xv2", [OUT, NV_PAD], bf, kind="ExternalOutput")
    out_s = nc.dram_tensor("xs2", [OUT, NS_PAD], bf, kind="ExternalOutput")

    with tile.TileContext(nc) as tc:
        with (
            tc.tile_pool(name="const", bufs=1) as cpool,
            tc.tile_pool(name="upool", bufs=3) as upool,
            tc.tile_pool(name="msg", bufs=8) as mpool,
            tc.tile_pool(name="big", bufs=1) as bpool,
            tc.tile_pool(name="zp", bufs=3, space="PSUM") as zpool,
            tc.tile_pool(name="accp", bufs=2, space="PSUM") as apool,
        ):
            C = {}
            for k in dt_in:
                if k.startswith('U_'):
                    continue
                sh, d = dt_in[k]
                t = cpool.tile(sh, d, tag=k)
                nc.sync.dma_start(out=t[:], in_=dram[k][:])
                C[k] = t

            oT_v = bpool.tile([OUT, NV_PAD], bf, tag="oTv")
            oT_s = bpool.tile([OUT, NS_PAD], bf, tag="oTs")

            relu_i = [0]
            copy_i = [0]

            def edge_pass(G, ntiles, udram, w1key, wl2key, oT):
                # burst list: (tile, col_off, ngroups, first_in_tile,
                #              last_in_tile)
                bursts = []
                off = 0
                for t in range(ntiles):
                    g = int(G[t])
                    done = 0
                    while done < g:
                        ng = min(BURST_G, g - done)
                        bursts.append((t, off + done * TO, ng,
                                       done == 0, done + ng == g))
                        done += ng
                    off += g * TO
                # U chunk spans (col_off, ncols), within tiles
                uchunks = []
                off = 0
                for t in range(ntiles):
                    g = int(G[t])
                    done = 0
                    while done < g:
                        ng = min(CHUNK_G, g - done)
                        uchunks.append((off + done * TO, ng * TO))
                        done += ng
                    off += g * TO
                chunk_tiles = {}
                ci = [0]

                def chunk_for(col_off, width):
                    while not (uchunks[ci[0]][0] <= col_off
                               and col_off + width
                               <= uchunks[ci[0]][0] + uchunks[ci[0]][1]):
                        ci[0] += 1
                    if ci[0] not in chunk_tiles:
                        co, cw = uchunks[ci[0]]
                        u = upool.tile([2 * KF, CHUNK_G * TO], f8, tag="u")
                        nc.sync.dma_start(out=u[:, :cw],
                                          in_=udram[:, co:co + cw])
                        chunk_tiles[ci[0]] = (u, co)
                    return chunk_tiles[ci[0]]

                acc = {}

                def mm1_burst(binfo):
                    t, coff, ng, first, last = binfo
                    u, co = chunk_for(coff, ng * TO)
                    msgs = []
                    for z0 in range(0, ng, 2):
                        nz = min(2, ng - z0)
                        ps = zpool.tile([128, 2 * TO], f32, space="PSUM",
                                        name="z", tag="z")
                        for j in range(nz):
                            nc.tensor.matmul(
                                out=ps[:, j * TO:(j + 1) * TO],
                                lhsT=C[w1key][:],
                                rhs=u[:, coff - co + (z0 + j) * TO:
                                      coff - co + (z0 + j + 1) * TO],
                                start=True, stop=True)
                        msg = mpool.tile([128, 2 * TO], bf, tag="m")
                        if relu_i[0] % 9 < 5:
                            nc.scalar.activation(
                                out=msg[:, :nz * TO],
                                in_=ps[:, :nz * TO], func=Relu)
                        else:
                            nc.vector.tensor_scalar_max(
                                out=msg[:, :nz * TO],
                                in0=ps[:, :nz * TO], scalar1=0.0)
                        relu_i[0] += 1
                        msgs.append((msg, nz))
                    return msgs

                def mm2_burst(binfo, msgs):
                    t, coff, ng, first, last = binfo
                    if first:
                        acc[t] = apool.tile([OUT, TO], f32, space="PSUM",
                                            name="acc", tag="acc")
                    a = acc[t]
                    j2 = 0
                    for msg, nz in msgs:
                        for j in range(nz):
                            j2 += 1
                            nc.tensor.matmul(
                                out=a[:], lhsT=C[wl2key][:],
                                rhs=msg[:, j * TO:(j + 1) * TO],
                                start=(first and j2 == 1),
                                stop=(last and j2 == ng),
                                skip_group_check=True)
                    if last:
                        if copy_i[0] % 2 == 0:
                            nc.vector.tensor_copy(
                                out=oT[:, t * TO:(t + 1) * TO], in_=a[:])
                        else:
                            nc.scalar.activation(
                                out=oT[:, t * TO:(t + 1) * TO], in_=a[:],
                                func=Copy)
                        copy_i[0] += 1
                        del acc[t]

                prev = None
                for binfo in bursts:
                    msgs = mm1_burst(binfo)
                    if prev is not None:
                        mm2_burst(*prev)
                    prev = (binfo, msgs)
                mm2_burst(*prev)

            edge_pass(G_A, NT_A, dram['U_A'], 'W1Astk', 'Wl2Astk', oT_v)
            edge_pass(G_B, NT_B, dram['U_B'], 'W1Bstk', 'Wl2Bstk', oT_s)
            nc.sync.dma_start(out=out_v[:], in_=oT_v[:])
            nc.sync.dma_start(out=out_s[:], in_=oT_s[:])

    nc.compile()
    return nc


def _in_maps(dev, shared):
    base = {k: np.asarray(shared[k]) for k in
            ('W1Astk', 'W1Bstk', 'Wl2Astk', 'Wl2Bstk')}
    maps = []
    for c in range(M):
        m = dict(base)
        m.update(U_A=dev[c]['U_A'], U_B=dev[c]['U_B'])
        maps.append(m)
    return maps


_CACHE = {}


def kernel(**inputs):
    import sys
    for p in ("/opt/trn_rl_repo",):
        if p not in sys.path:
            sys.path.insert(0, p)
    from concourse.bass_utils import run_bass_kernel_spmd

    W = {k: np.asarray(v, np.float32) for k, v in inputs.items()
         if k[0] in ('W', 'b')}
    dev, shared, meta = _prep(inputs['x_site'], inputs['x_vendor'],
                              inputs['src'], inputs['dst'], W)
    key = (tuple(shared['G_A'].tolist()), tuple(shared['G_B'].tolist()))
    if key not in _CACHE:
        _CACHE[key] = build_bass(shared)
    nc = _CACHE[key]
    res = run_bass_kernel_spmd(nc, _in_maps(dev, shared), list(range(M)))

    out = np.zeros((NS + NV, OUT), np.float32)
    so, sl = meta['s_owner'], meta['s_local']
    vo, vl = meta['v_owner'], meta['v_local']
    for c in range(M):
        S_s = np.asarray(res.results[c]['xs2'].T, np.float32)  # [NS_PAD,32]
        S_v = np.asarray(res.results[c]['xv2'].T, np.float32)
        sel = np.flatnonzero(so == c)
        out[sel] = np.maximum(
            S_s[sl[sel]] * meta['rs'][sel][:, None] + meta['T_s'][sel], 0)
        sel = np.flatnonzero(vo == c)
        out[NS + sel] = np.maximum(
            S_v[vl[sel]] * meta['rv'][sel][:, None] + meta['T_v'][sel], 0)
    return out
